# revision 50
# baseline (speedup 1.0000x reference)
"""AttentionLSTM Trainium2 kernel (8-core SPMD, data-parallel over batch).

Problem: N=256, T=128, D=512, H=1024.
    h0 = c0 = mean(A, (2,3));  per step:
      M = einsum('nh,nhk->nk', h, A2)/sqrt(H); w = softmax(M)
      attn = einsum('nhk,nk->nh', A2, w)
      act = x_t@Wx + h@Wh + attn@Wattn + b -> i,f,o,g -> LSTM update

Per-core design (32 batch rows):
  - All recurrent matmuls in bf16, accumulated in fp32 PSUM, with PE
    column-tiling (tile_position=(0,32q)) so 4 independent M=32 matmuls
    stream concurrently.
  - attn@Wattn is algebraically folded: P[(n,k),:] = A2[n,:,k]@Wattn is
    precomputed once (f32r matmuls); per step act += wBD.T @ P where wBD is
    the block-diagonal softmax weights - attn itself never materializes.
    All psA (i/f gate) attention chunks run before psB chunks so the gate
    activations start while attention still streams.
  - M-phase uses the same diag trick: psum_M = hT.T @ A2sb (+ additive
    block-diagonal -1e30 mask via an identity matmul, issued first so it
    can slot into the previous step's gate window); one Exp activation
    with accum_out yields both exp(M/32) and its row-sum.
  - x@Wx (+b) is precomputed to DRAM in bf16; the per-step one-hot matmul
    that injects it carries start=True, doubling as the psum-open. psA/psB
    double-buffer across steps so these opens don't WAR the gate reads.
  - Gates: sigmoid(x) = 0.5+0.5*tanh(x/2) keeps the ACT engine on the
    exp/tanh table set (no ACT_TABLE_LOAD swaps); the i/f affines fold
    into fused scalar_tensor_tensor ops by carrying the cell state
    doubled (cfull = 2c); o gets an explicit affine so h is exact.
  - State update is column-half pipelined: half 0's [64,128] PE
    transposes (paired h-blocks j/j+4) and the next step's matmuls start
    while half 1's DVE/ACT chain still runs. hT lives in two per-half
    tiles (hTa/hTb).
  - Small keep-warm matmuls pinned to the DVE chain limit HAM re-throttle
    damage across the gate window.
"""
import math
from contextlib import ExitStack

import numpy as np
import ml_dtypes

import concourse.bass as bass
import concourse.mybir as mybir
import concourse.tile as tile
from concourse.bass import ts
from concourse.bass_utils import run_bass_kernel_spmd
from concourse.vector_clock import ScopedClock

dt = mybir.dt
AF = mybir.ActivationFunctionType
ALU = mybir.AluOpType

N, T, D, H = 256, 128, 512, 1024
NCORES = 8
NL = N // NCORES          # 32 batch rows per core
G = 4 * H                 # 4096 gate columns
NK = NL * 16              # 512 (n,k) pairs
SCALE = 1.0 / math.sqrt(H)


class PatchedTileContext(tile.TileContext):
    """This walrus build allows at most one sem wait per SP TPB_CTRL
    instruction; put the tail waits on single-wait NoOps before the drain."""

    def _drain_and_barrier(self, tick_clock, wait_clock):
        collector = self.nc.sync.nop(nofuse=True, hint="tail_waits")
        wait_clock.add_sem_waits(
            collector.ins, ScopedClock({None: tick_clock.global_clock})
        )
        waits = list(collector.ins.sync_info.on_wait) if collector.ins.sync_info else []
        collector.ins.sync_info = mybir.SyncInfo(on_wait=waits[:1], on_update=[])
        for w in waits[1:]:
            n = self.nc.sync.nop(nofuse=True, hint="tail_waits")
            n.ins.sync_info = mybir.SyncInfo(on_wait=[w], on_update=[])
        self.nc.sync.drain()
        self.nc.all_engine_barrier()
        assert self.sems is not None
        popped = self.nc._tile_sem_poison_stack.pop()
        assert popped is self._sem_poison
        self.nc.clear_and_free_semaphores(list(self.sems.allocated().values()))
        self.nc.all_engine_barrier()


def split_multi_waits(nc):
    """Walrus here rejects >1 sem wait per instruction: move extras onto
    same-engine NoOps inserted just before the instruction."""
    for f in nc.m.functions:
        for bb in f.blocks:
            new_insts = []
            for inst in bb.instructions:
                si = inst.sync_info
                if si is not None and len(si.on_wait) > 1:
                    waits = list(si.on_wait)
                    for w in waits[:-1]:
                        n = mybir.InstNoOp(
                            name=nc.get_next_instruction_name(),
                            engine=inst.engine,
                            ins=[],
                            outs=[],
                            sync_info=mybir.SyncInfo(on_wait=[w], on_update=[]),
                        )
                        new_insts.append(n)
                    inst.sync_info = mybir.SyncInfo(
                        on_wait=[waits[-1]], on_update=list(si.on_update)
                    )
                new_insts.append(inst)
            try:
                bb.instructions[:] = new_insts
            except TypeError:
                bb.instructions = new_insts


def _np_bf16(a):
    return a.astype(ml_dtypes.bfloat16)


def build(t_steps=T, split=True, reps=1, ablate=()):
    nc = bass.Bass("TRN2", target_bir_lowering=False, debug=False, num_devices=NCORES)

    x_d = nc.dram_tensor("x", [NL, T, D], dt.float32, kind="ExternalInput")
    A_d = nc.dram_tensor("A", [NL, H, 16], dt.float32, kind="ExternalInput")
    Wx_d = nc.dram_tensor("Wx", [D, G], dt.float32, kind="ExternalInput")
    Wh_d = nc.dram_tensor("Wh", [H, G], dt.float32, kind="ExternalInput")
    Wattn_d = nc.dram_tensor("Wattn", [H, G], dt.float32, kind="ExternalInput")
    b_d = nc.dram_tensor("b", [1, G], dt.float32, kind="ExternalInput")
    out_d = nc.dram_tensor("out", [NL, T, H], dt.float32, kind="ExternalOutput")
    # last row of each = bf16 hi/lo of the bias b
    xhi_d = nc.dram_tensor("xhi", [NL * T + 1, G], dt.bfloat16, kind="Internal")

    # ---- inline constants
    mask_np = np.full((NL, NK), -1e30, np.float32)
    for n in range(NL):
        mask_np[n, 16 * n : 16 * n + 16] = 0.0
    mask_c = nc.inline_tensor(_np_bf16(mask_np), name="maskbd")
    e33_np = np.zeros((NL + 1, NL), np.float32)
    e33_np[:NL, :NL] = np.eye(NL)
    e33_np[NL, :] = 1.0
    e33_c = nc.inline_tensor(_np_bf16(e33_np), name="e33")
    id64_c = nc.inline_tensor(np.tile(np.eye(32, dtype=np.float32), (2, 1)), name="id64")
    id64x64_c = nc.inline_tensor(np.eye(64, dtype=np.float32), name="id64x64")
    # per-partition tanh scale for psB: rows 0-63 (g) scale 1, rows 64-127 (o) 0.5
    svec_np = np.ones((128, 1), np.float32)
    svec_np[64:128] = 0.5
    svec_c = nc.inline_tensor(svec_np, name="svec")
    z128_c = nc.inline_tensor(np.zeros((1, 128), ml_dtypes.bfloat16), name="z128")
    id128_c = nc.inline_tensor(np.eye(128, dtype=np.float32), name="id128")

    # hT column offset of h-block j under the paired [64,128] transpose layout
    def off(j):
        return 64 * (j % 4) + 32 * (j // 4)

    with PatchedTileContext(nc) as tc, ExitStack() as stack:
        persist = stack.enter_context(tc.tile_pool(name="persist", bufs=1))
        Psb = [persist.tile([128, G], dt.bfloat16, tag=f"p{m}", name=f"p{m}") for m in range(4)]
        A2bf = [persist.tile([128, NK], dt.bfloat16, tag=f"a2b{j}", name=f"a2b{j}") for j in range(8)]
        # hT split into per-half tiles so next-step matmuls can start on the
        # first transposed half while the second half's state ops still run
        hTab = [
            persist.tile([128, 128], dt.bfloat16, tag="hTa", name="hTa"),
            persist.tile([128, 128], dt.bfloat16, tag="hTb", name="hTb"),
        ]
        cfull = persist.tile([128, 512], dt.float32, tag="cfull", name="cfull")

        def hsl(j):
            """lhsT slice of h-block j: tile hx=(j%4)//2, col 64*(j%4-2*hx)+32*(j//4)."""
            hx = (j % 4) // 2
            c = 64 * (j % 4 - 2 * hx) + 32 * (j // 4)
            return hTab[hx][:, c : c + 32]

        # ============ Phase B: A2 layouts, h0T, c0, P = A2^T @ Wattn
        with (
            tc.tile_pool(name="pcb1", bufs=1) as pcb1,
            tc.tile_pool(name="pcb2", bufs=2) as pcb2,
            tc.tile_pool(name="pcbps", bufs=3, space="PSUM") as pcbps,
        ):
            A_hnk = A_d[:, :, :].rearrange("n h k -> h n k")  # [H, NL, 16]
            A2r = [pcb1.tile([128, NK], dt.float32r, tag=f"a2r{j}", name=f"a2r{j}") for j in range(8)]
            h0scr = pcb1.tile([128, 32], dt.float32, tag="h0scr", name="h0scr")
            for j in range(8):
                nc.sync.dma_start(
                    out=A2r[j].rearrange("h (n k) -> h n k", k=16),
                    in_=A_hnk[ts(j, 128), :, :].bitcast(dt.float32r),
                )
                nc.vector.tensor_copy(A2bf[j], A2r[j].bitcast(dt.float32))
                nc.vector.tensor_reduce(
                    h0scr,
                    A2r[j].bitcast(dt.float32).rearrange("h (n k) -> h n k", k=16),
                    mybir.AxisListType.X,
                    ALU.add,
                )
                nc.scalar.mul(hsl(j), h0scr, 1.0 / 16.0)
            # c0 stacked into cfull[64:128], 8 h-slices of 128
            for qq in range(8):
                a2n = pcb2.tile([NL, 128 * 16], dt.float32, tag="a2n", name="a2n")
                nc.sync.dma_start(
                    out=a2n.rearrange("n (h k) -> n h k", k=16),
                    in_=A_d[:, ts(qq, 128), :],
                )
                c0scr = pcb2.tile([NL, 128], dt.float32, tag="c0scr", name="c0scr")
                nc.vector.tensor_reduce(
                    c0scr,
                    a2n.rearrange("n (h k) -> n h k", k=16),
                    mybir.AxisListType.X,
                    ALU.add,
                )
                q, r2 = qq // 4, qq % 4
                nc.scalar.mul(
                    cfull[64 + 32 * q : 96 + 32 * q, ts(r2, 128)], c0scr, 1.0 / 8.0
                )
            # P in two Wattn halves; second half added in place (bf16)
            wat = [pcb1.tile([128, G], dt.float32r, tag=f"wat{j}", name=f"wat{j}") for j in range(4)]
            for half in range(2):
                for j in range(4):
                    nc.sync.dma_start(
                        out=wat[j],
                        in_=Wattn_d[ts(4 * half + j, 128), :].bitcast(dt.float32r),
                    )
                for m in range(4):
                    for c in range(8):
                        pps = pcbps.tile([128, 512], dt.float32, tag="pps", name="pps")
                        for j in range(4):
                            nc.tensor.matmul(
                                pps,
                                A2r[4 * half + j][:, ts(m, 128)],
                                wat[j][:, ts(c, 512)],
                                start=(j == 0),
                                stop=(j == 3),
                            )
                        if half == 0:
                            nc.scalar.copy(Psb[m][:, ts(c, 512)], pps)
                        else:
                            nc.vector.tensor_add(
                                Psb[m][:, ts(c, 512)], pps, Psb[m][:, ts(c, 512)]
                            )

        # ============ Phase A: xact = x @ Wx -> DRAM bf16 hi/lo (+ b row)
        with tc.tile_pool(name="bpool", bufs=1) as bpool:
            b_f = bpool.tile([1, G], dt.float32, tag="b_f", name="b_f")
            nc.sync.dma_start(out=b_f, in_=b_d[:, :])
            bhi = bpool.tile([1, G], dt.bfloat16, tag="bhi", name="bhi")
            nc.vector.tensor_copy(bhi, b_f)
            nc.sync.dma_start(out=xhi_d[NL * T : NL * T + 1, :], in_=bhi)

        with (
            tc.tile_pool(name="pca1", bufs=1) as pca1,
            tc.tile_pool(name="pca", bufs=2) as pca,
            tc.tile_pool(name="pcaps", bufs=3, space="PSUM") as pcaps,
        ):
            id128r = pca1.tile([128, 128], dt.float32r, tag="id128r", name="id128r")
            nc.sync.dma_start(out=id128r, in_=id128_c[:, :].bitcast(dt.float32r))
            Wxsb = [pca1.tile([128, G], dt.float32r, tag=f"wx{k}", name=f"wx{k}") for k in range(4)]
            for k in range(4):
                nc.sync.dma_start(
                    out=Wxsb[k], in_=Wx_d[ts(k, 128), :].bitcast(dt.float32r)
                )

            for i in range(NL):
                xsb = pca.tile([128, D], dt.float32r, tag="xsb", name="xsb")
                nc.sync.dma_start(out=xsb, in_=x_d[i, :, :].bitcast(dt.float32r))
                xt_ps = pcaps.tile([128, 512], dt.float32r, tag="xtps", name="xtps")
                for k in range(4):
                    nc.tensor.transpose(xt_ps[:, ts(k, 128)], xsb[:, ts(k, 128)], id128r)
                xT = pca.tile([128, 512], dt.float32r, tag="xT", name="xT")
                nc.scalar.copy(xT, xt_ps)
                for c in range(8):
                    aps = pcaps.tile([128, 512], dt.float32, tag="acps", name="acps")
                    for k in range(4):
                        nc.tensor.matmul(
                            aps,
                            xT[:, ts(k, 128)],
                            Wxsb[k][:, ts(c, 512)],
                            start=(k == 0),
                            stop=(k == 3),
                        )
                    hi = pca.tile([128, 512], dt.bfloat16, tag="hi", name="hi")
                    nc.scalar.copy(hi, aps)
                    nc.sync.dma_start(out=xhi_d[ts(i, 128), ts(c, 512)], in_=hi)

        # ============ Phase C: Wh -> bf16 resident
        whpool = stack.enter_context(tc.tile_pool(name="whpool", bufs=1))
        Whsb = [whpool.tile([128, G], dt.bfloat16, tag=f"wh{j}", name=f"wh{j}") for j in range(8)]
        with tc.tile_pool(name="pcc", bufs=2) as pcc:
            for j in range(8):
                stage = pcc.tile([128, G], dt.float32, tag="whstage", name="whstage")
                nc.sync.dma_start(out=stage, in_=Wh_d[ts(j, 128), :])
                nc.vector.tensor_copy(Whsb[j], stage)

        # ============ Recurrent loop
        loopp = stack.enter_context(tc.tile_pool(name="loopp", bufs=1))
        sbIF = loopp.tile([128, 512], dt.float32, tag="sbIF", name="sbIF")
        tB = loopp.tile([128, 512], dt.float32, tag="tB", name="tB")
        th_sb = loopp.tile([128, 512], dt.float32, tag="th_sb", name="th_sb")  # rows 64-127 used (STT base-partition rule)
        o_sb = loopp.tile([128, 512], dt.float32, tag="o_sb", name="o_sb")  # rows 64-127 used
        u_sb = loopp.tile([64, 512], dt.float32, tag="u_sb", name="u_sb")
        v_sb = loopp.tile([64, 512], dt.float32, tag="v_sb", name="v_sb")
        h_sb = loopp.tile([64, 512], dt.float32, tag="h_sb", name="h_sb")
        wsum = loopp.tile([32, 256], dt.float32, tag="wsum", name="wsum")
        svec = loopp.tile([128, 1], dt.float32, tag="svec", name="svec")
        id64x64 = loopp.tile([64, 64], dt.float32, tag="id64x64", name="id64x64")
        nc.sync.dma_start(out=svec, in_=svec_c[:, :])
        nc.sync.dma_start(out=id64x64, in_=id64x64_c[:, :])
        wexp = loopp.tile([NL, NK], dt.float32, tag="wexp", name="wexp")
        wexpn = loopp.tile([NL, NK], dt.float32, tag="wexpn", name="wexpn")
        s_sb = loopp.tile([NL, 1], dt.float32, tag="s_sb", name="s_sb")
        rs_sb = loopp.tile([NL, 1], dt.float32, tag="rs_sb", name="rs_sb")
        wBD = loopp.tile([128, 128], dt.bfloat16, tag="wBD", name="wBD")
        maskbf = loopp.tile([NL, NK], dt.bfloat16, tag="maskbf", name="maskbf")
        e33 = loopp.tile([NL + 1, NL], dt.bfloat16, tag="e33t", name="e33t")
        id64 = loopp.tile([64, 32], dt.float32, tag="id64t", name="id64t")
        z128 = loopp.tile([1, 128], dt.bfloat16, tag="z128t", name="z128t")
        nc.sync.dma_start(out=z128, in_=z128_c[:, :])
        nc.sync.dma_start(out=maskbf, in_=mask_c[:, :])
        nc.sync.dma_start(out=e33, in_=e33_c[:, :])
        nc.sync.dma_start(out=id64, in_=id64_c[:, :])

        lps = stack.enter_context(tc.tile_pool(name="lps", bufs=1, space="PSUM"))
        xpool = stack.enter_context(tc.tile_pool(name="xpool", bufs=2))
        xhi_nt = xhi_d[0 : NL * T, :].rearrange("(n t) g -> n t g", t=T)

        # chunk -> (psum tensor, block q, gate column slice)
        # psA blocks: (i,0) (i,1) (f,0) (f,1); psB blocks: (g,0) (g,1) (o,0) (o,1)
        chunk_cols = {}
        for q, (gb, half) in enumerate([(0, 0), (0, 1), (H, 0), (H, 1)]):
            chunk_cols[("A", q)] = slice(gb + 512 * half, gb + 512 * half + 512)
        for q, (gb, half) in enumerate([(3 * H, 0), (3 * H, 1), (2 * H, 0), (2 * H, 1)]):
            chunk_cols[("B", q)] = slice(gb + 512 * half, gb + 512 * half + 512)

        rep_ctx = tc.For_i(0, reps, 1) if reps > 1 else None
        if rep_ctx is not None:
            rep_ctx.__enter__()

        def make_xact(t):
            """Prefetch x-act tiles for step t and open each psum chunk with
            its one-hot x-act matmul (start=True zeroes the chunk rows; runs
            in the previous step's tail, keeping PE warm). psA/psB alternate
            between two psum banks so these opens don't WAR against the
            previous step's gate reads."""
            xhi_t = xpool.tile([NL + 1, G], dt.bfloat16, tag="xhi", name="xhi")
            nc.sync.dma_start(out=xhi_t[0:NL, :], in_=xhi_nt[:, t, :])
            nc.sync.dma_start(out=xhi_t[NL : NL + 1, :], in_=xhi_d[NL * T : NL * T + 1, :])
            pb = t % 2
            psA = lps.tile([128, 512], dt.float32, tag=f"psA{pb}", name=f"psA{pb}")
            psB = lps.tile([128, 512], dt.float32, tag=f"psB{pb}", name=f"psB{pb}")
            plan = []
            for which, ps in (("A", psA), ("B", psB)):
                for q in range(4):
                    plan.append((ps[ts(q, 32), :], (0, 32 * q), chunk_cols[(which, q)]))
            order = [0, 4, 1, 5, 2, 6, 3, 7]
            for ci in order:
                dst, tp, cs = plan[ci]
                nc.tensor.matmul(dst, e33, xhi_t[:, cs], start=True, stop=False, tile_position=tp)
            return psA, psB, plan

        nc.vector.memset(wBD, 0.0)
        cur = make_xact(0)
        # bank budget (8): psA0 psB0 psA1 psB1 psM + these three.
        # psWT shares its bank with the warm scratch (both only PE-written
        # before their single reader); psHTa/psHTb get their own banks so
        # the tail transposes/casts don't serialize on bank-overlap pairs.
        psWT_ps = lps.tile([128, 512], dt.float32, tag="psWT", name="psWT")
        warm_ps = psWT_ps[0:32, 128:256]
        psHTa = lps.tile([128, 128], dt.float32, tag="psHTa", name="psHTa")
        psHTb = lps.tile([128, 128], dt.float32, tag="psHTb", name="psHTb")

        for t in range(t_steps):
            psA, psB, plan = cur

            # ---- M-phase [32,512] on strip 0, h@Wh on strips 1-3. The mask
            # matmul runs FIRST (start=True) — it has no hT dependency, so the
            # scheduler can slot it into the previous step's gate window.
            # j-order consumes hTa then hTb, chasing the half-transposes.
            psM = lps.tile([NL, NK], dt.float32, tag="psM", name="psM")
            c123 = [1, 2, 3, 5, 6, 7]
            JORD = [0, 4, 1, 5, 2, 6, 3, 7]
            nc.tensor.matmul(psM, e33[0:NL, :], maskbf, start=True, stop=False)
            for idx, j in enumerate(JORD):
                hs = hsl(j)
                nc.tensor.matmul(psM, hs, A2bf[j], start=False, stop=(idx == 7))
                for ci in (c123[3 * (idx % 2)], c123[3 * (idx % 2) + 1], c123[3 * (idx % 2) + 2]):
                    dst, tp, cs = plan[ci]
                    nc.tensor.matmul(dst, hs, Whsb[j][:, cs], start=False, stop=False, tile_position=tp)
            # remaining h@Wh rounds (overlap the softmax chain)
            done = {(c123[3 * (idx % 2) + r], JORD[idx]) for idx in range(8) for r in range(3)}
            rest = [(ci, j) for j in range(8) for ci in [0, 4, 1, 5, 2, 6, 3, 7] if (ci, j) not in done]
            # round-robin the remainder in hTa-first order to keep streams busy
            rest.sort(key=lambda cj: (JORD.index(cj[1]), cj[0]))
            for ci, j in rest:
                dst, tp, cs = plan[ci]
                nc.tensor.matmul(dst, hsl(j), Whsb[j][:, cs], start=False, stop=False, tile_position=tp)

            # ---- softmax
            if "softmax" not in ablate:
                nc.scalar.activation(wexp, psM, AF.Exp, scale=SCALE, accum_out=s_sb)
                nc.vector.reciprocal(rs_sb, s_sb)
                nc.vector.tensor_scalar_mul(wexpn, wexp, rs_sb)
                # ---- wBD (PE transposes of normalized weights)
                psWT = psWT_ps[:, 0:128]
                for m in range(4):
                    nc.tensor.transpose(psWT[:, ts(m, 32)], wexpn[:, ts(m, 128)], id64[0:32, :])
                nc.vector.tensor_copy(wBD, psWT)

            # ---- act matmuls part 2 (attention via P). All psA chunks run
            # first (then psA's stop), so the i/f gate tanh starts while the
            # psB half of attention still streams.
            if "attn" not in ablate:
                for m in range(4):
                    for ci in (0, 1, 2, 3):
                        dst, tp, cs = plan[ci]
                        nc.tensor.matmul(dst, wBD[:, ts(m, 32)], Psb[m][:, cs], start=False, stop=False, tile_position=tp)
                nc.tensor.matmul(psA[:, 0:1], z128, maskbf[0:1, 0:1], start=False, stop=True)
                for m in range(4):
                    for ci in (4, 5, 6, 7):
                        dst, tp, cs = plan[ci]
                        nc.tensor.matmul(dst, wBD[:, ts(m, 32)], Psb[m][:, cs], start=False, stop=False, tile_position=tp)
                nc.tensor.matmul(psB[:, 0:1], z128, maskbf[0:1, 0:1], start=False, stop=True)
            else:
                nc.tensor.matmul(psA[:, 0:1], z128, maskbf[0:1, 0:1], start=False, stop=True)
                nc.tensor.matmul(psB[:, 0:1], z128, maskbf[0:1, 0:1], start=False, stop=True)

            # ---- gates + state update, column-half pipelined.
            # sigmoid via tanh (sigma(x)=0.5+0.5*tanh(x/2)) keeps ACT on one
            # table set; the i/f affines fold into STT ops by carrying the
            # cell state doubled (cfull holds C=2c); o gets an explicit ACT
            # affine so h comes out exact (DMA'd directly).
            # Half hx covers columns 256hx:256hx+256 of every [*,512] state
            # tile, so the hx=0 transposes + next-step matmuls overlap hx=1.
            H0, H1 = slice(0, 256), slice(256, 512)
            # psA-only reads first (start during attention's psB half);
            # v = f*C also needs only psA, so it leads the DVE queue.
            nc.scalar.activation(sbIF[:, H0], psA[:, H0], AF.Tanh, scale=0.5)  # [ti; tf]
            nc.scalar.activation(sbIF[:, H1], psA[:, H1], AF.Tanh, scale=0.5)
            nc.scalar.activation(tB[:, H0], psB[:, H0], AF.Tanh, scale=svec)   # [g; to]
            nc.scalar.activation(tB[:, H1], psB[:, H1], AF.Tanh, scale=svec)
            nc.vector.scalar_tensor_tensor(v_sb[:, H0], sbIF[64:128, H0], 1.0, cfull[64:128, H0], op0=ALU.add, op1=ALU.mult)
            nc.vector.scalar_tensor_tensor(v_sb[:, H1], sbIF[64:128, H1], 1.0, cfull[64:128, H1], op0=ALU.add, op1=ALU.mult)
            # next step's x-act prefetch + psum-open runs in this tail
            if t + 1 < t_steps:
                nxt = make_xact(t + 1)
            else:
                nxt = None
            # keep-warm matmuls pinned to the DVE chain: the gate window is
            # ~4.5us of PE idle, and HAM only holds K=8/8 if the PE stays
            # busy; five N=512 warm streams (~2us busy) keep the duty cycle
            # high enough that the next step's matmul head runs at 2.4 GHz
            if "warm" not in ablate:
                nc.tensor.matmul(warm_ps, id64[0:32, :], v_sb[0:32, 0:128], start=True, stop=True)
            nc.scalar.activation(o_sb[64:128, H0], tB[64:128, H0], AF.Copy, bias=0.5, scale=0.5)
            nc.scalar.activation(o_sb[64:128, H1], tB[64:128, H1], AF.Copy, bias=0.5, scale=0.5)
            # u2 = (1+ti)*g = 2u ; C_new = u2 + 0.5*v2
            nc.vector.scalar_tensor_tensor(u_sb[:, H0], sbIF[0:64, H0], 1.0, tB[0:64, H0], op0=ALU.add, op1=ALU.mult)
            nc.vector.scalar_tensor_tensor(cfull[64:128, H0], v_sb[:, H0], 0.5, u_sb[:, H0], op0=ALU.mult, op1=ALU.add)
            nc.scalar.activation(th_sb[64:128, H0], cfull[64:128, H0], AF.Tanh, scale=0.5)
            nc.vector.scalar_tensor_tensor(u_sb[:, H1], sbIF[0:64, H1], 1.0, tB[0:64, H1], op0=ALU.add, op1=ALU.mult)
            nc.vector.scalar_tensor_tensor(cfull[64:128, H1], v_sb[:, H1], 0.5, u_sb[:, H1], op0=ALU.mult, op1=ALU.add)
            if "warm" not in ablate:
                nc.tensor.matmul(warm_ps, id64[0:32, :], u_sb[0:32, 0:128], start=True, stop=True)
            # h = o*th  [64,512]: rows 0-31 h cols 0:512, rows 32-63 h cols 512:1024
            nc.vector.tensor_mul(h_sb[:, H0], o_sb[64:128, H0], th_sb[64:128, H0])
            if t + 1 < t_steps:
                nc.tensor.transpose(psHTa[:, 0:64], h_sb[:, 0:128], id64x64)
                nc.tensor.transpose(psHTa[:, 64:128], h_sb[:, 128:256], id64x64)
            nc.scalar.activation(th_sb[64:128, H1], cfull[64:128, H1], AF.Tanh, scale=0.5)
            if t + 1 < t_steps:
                nc.vector.tensor_copy(hTab[0], psHTa)
            nc.vector.tensor_mul(h_sb[:, H1], o_sb[64:128, H1], th_sb[64:128, H1])
            if "warm" not in ablate:
                nc.tensor.matmul(warm_ps, id64[0:32, :], h_sb[0:32, 0:128], start=True, stop=True)
            if t + 1 < t_steps:
                nc.tensor.transpose(psHTb[:, 0:64], h_sb[:, 256:384], id64x64)
                nc.tensor.transpose(psHTb[:, 64:128], h_sb[:, 384:512], id64x64)
                nc.vector.tensor_copy(hTab[1], psHTb)
            nc.sync.dma_start(out=out_d[:, t, 0:512], in_=h_sb[0:32, :])
            nc.sync.dma_start(out=out_d[:, t, 512:1024], in_=h_sb[32:64, :])
            cur = nxt
        if rep_ctx is not None:
            rep_ctx.__exit__(None, None, None)
    if split:
        split_multi_waits(nc)
    return nc


_CACHE = {}


def _get_nc(t_steps):
    if t_steps not in _CACHE:
        _CACHE[t_steps] = build(t_steps)
    return _CACHE[t_steps]


def kernel(x, A, Wx, Wh, Wattn, b, t_steps=T, trace=False):
    x = np.asarray(x, np.float32)
    A = np.asarray(A, np.float32).reshape(N, H, 16)
    Wx = np.ascontiguousarray(np.asarray(Wx, np.float32))
    Wh = np.ascontiguousarray(np.asarray(Wh, np.float32))
    Wattn = np.ascontiguousarray(np.asarray(Wattn, np.float32))
    b = np.asarray(b, np.float32).reshape(1, G)

    nc = _get_nc(t_steps)
    in_maps = []
    for c in range(NCORES):
        sl = slice(NL * c, NL * (c + 1))
        in_maps.append(
            {
                "x": np.ascontiguousarray(x[sl]),
                "A": np.ascontiguousarray(A[sl]),
                "Wx": Wx,
                "Wh": Wh,
                "Wattn": Wattn,
                "b": b,
            }
        )
    res = run_bass_kernel_spmd(nc, in_maps, core_ids=list(range(NCORES)), trace=trace)
    out = np.concatenate([r["out"] for r in res.results], axis=0)
    if trace:
        kernel.last_exec_time_ns = res.exec_time_ns
    return out


kernel.last_exec_time_ns = None



# revision 51
# speedup vs baseline: 1.1474x; 1.1474x over previous
"""AttentionLSTM Trainium2 kernel (8-core SPMD, data-parallel over batch).

Problem: N=256, T=128, D=512, H=1024.
    h0 = c0 = mean(A, (2,3));  per step:
      M = einsum('nh,nhk->nk', h, A2)/sqrt(H); w = softmax(M)
      attn = einsum('nhk,nk->nh', A2, w)
      act = x_t@Wx + h@Wh + attn@Wattn + b -> i,f,o,g -> LSTM update

Per-core design (32 batch rows):
  - All recurrent matmuls in bf16, accumulated in fp32 PSUM, with PE
    column-tiling (tile_position=(0,32q)) so 4 independent M=32 matmuls
    stream concurrently.
  - attn@Wattn is algebraically folded: P[(n,k),:] = A2[n,:,k]@Wattn is
    precomputed once (f32r matmuls); per step act += wBD.T @ P where wBD is
    the block-diagonal softmax weights - attn itself never materializes.
    All psA (i/f gate) attention chunks run before psB chunks so the gate
    activations start while attention still streams.
  - M-phase uses the same diag trick: psum_M = hT.T @ A2sb (+ additive
    block-diagonal -1e30 mask via an identity matmul, issued first so it
    can slot into the previous step's gate window); one Exp activation
    with accum_out yields both exp(M/32) and its row-sum.
  - x@Wx (+b) is precomputed to DRAM in bf16; the per-step one-hot matmul
    that injects it carries start=True, doubling as the psum-open. psA/psB
    double-buffer across steps so these opens don't WAR the gate reads.
  - Gates: sigmoid(x) = 0.5+0.5*tanh(x/2) keeps the ACT engine on the
    exp/tanh table set (no ACT_TABLE_LOAD swaps); the i/f affines fold
    into fused scalar_tensor_tensor ops by carrying the cell state
    doubled (cfull = 2c); o gets an explicit affine so h is exact.
  - State update is column-half pipelined: half 0's [64,128] PE
    transposes (paired h-blocks j/j+4) and the next step's matmuls start
    while half 1's DVE/ACT chain still runs. hT lives in two per-half
    tiles (hTa/hTb).
  - Small keep-warm matmuls pinned to the DVE chain limit HAM re-throttle
    damage across the gate window.
"""
import math
from contextlib import ExitStack

import numpy as np
import ml_dtypes

import concourse.bass as bass
import concourse.mybir as mybir
import concourse.tile as tile
from concourse.bass import ts
from concourse.bass_utils import run_bass_kernel_spmd
from concourse.vector_clock import ScopedClock

dt = mybir.dt
AF = mybir.ActivationFunctionType
ALU = mybir.AluOpType

N, T, D, H = 256, 128, 512, 1024
NCORES = 8
NL = N // NCORES          # 32 batch rows per core
G = 4 * H                 # 4096 gate columns
NK = NL * 16              # 512 (n,k) pairs
SCALE = 1.0 / math.sqrt(H)


class PatchedTileContext(tile.TileContext):
    """This walrus build allows at most one sem wait per SP TPB_CTRL
    instruction; put the tail waits on single-wait NoOps before the drain."""

    def _drain_and_barrier(self, tick_clock, wait_clock):
        collector = self.nc.sync.nop(nofuse=True, hint="tail_waits")
        wait_clock.add_sem_waits(
            collector.ins, ScopedClock({None: tick_clock.global_clock})
        )
        waits = list(collector.ins.sync_info.on_wait) if collector.ins.sync_info else []
        collector.ins.sync_info = mybir.SyncInfo(on_wait=waits[:1], on_update=[])
        for w in waits[1:]:
            n = self.nc.sync.nop(nofuse=True, hint="tail_waits")
            n.ins.sync_info = mybir.SyncInfo(on_wait=[w], on_update=[])
        self.nc.sync.drain()
        self.nc.all_engine_barrier()
        assert self.sems is not None
        popped = self.nc._tile_sem_poison_stack.pop()
        assert popped is self._sem_poison
        self.nc.clear_and_free_semaphores(list(self.sems.allocated().values()))
        self.nc.all_engine_barrier()


def split_multi_waits(nc):
    """Walrus here rejects >1 sem wait per instruction: move extras onto
    same-engine NoOps inserted just before the instruction."""
    for f in nc.m.functions:
        for bb in f.blocks:
            new_insts = []
            for inst in bb.instructions:
                si = inst.sync_info
                if si is not None and len(si.on_wait) > 1:
                    waits = list(si.on_wait)
                    for w in waits[:-1]:
                        n = mybir.InstNoOp(
                            name=nc.get_next_instruction_name(),
                            engine=inst.engine,
                            ins=[],
                            outs=[],
                            sync_info=mybir.SyncInfo(on_wait=[w], on_update=[]),
                        )
                        new_insts.append(n)
                    inst.sync_info = mybir.SyncInfo(
                        on_wait=[waits[-1]], on_update=list(si.on_update)
                    )
                new_insts.append(inst)
            try:
                bb.instructions[:] = new_insts
            except TypeError:
                bb.instructions = new_insts


def _np_bf16(a):
    return a.astype(ml_dtypes.bfloat16)


def build(t_steps=T, split=True, reps=1, ablate=()):
    nc = bass.Bass("TRN2", target_bir_lowering=False, debug=False, num_devices=NCORES)

    x_d = nc.dram_tensor("x", [NL, T, D], dt.float32, kind="ExternalInput")
    A_d = nc.dram_tensor("A", [NL, H, 16], dt.float32, kind="ExternalInput")
    Wx_d = nc.dram_tensor("Wx", [D, G], dt.float32, kind="ExternalInput")
    Wh_d = nc.dram_tensor("Wh", [H, G], dt.float32, kind="ExternalInput")
    Wattn_d = nc.dram_tensor("Wattn", [H, G], dt.float32, kind="ExternalInput")
    b_d = nc.dram_tensor("b", [1, G], dt.float32, kind="ExternalInput")
    out_d = nc.dram_tensor("out", [NL, T, H], dt.float32, kind="ExternalOutput")
    # last row of each = bf16 hi/lo of the bias b
    xhi_d = nc.dram_tensor("xhi", [NL * T + 1, G], dt.bfloat16, kind="Internal")

    # ---- inline constants
    mask_np = np.full((NL, NK), -1e30, np.float32)
    for n in range(NL):
        mask_np[n, 16 * n : 16 * n + 16] = 0.0
    mask_c = nc.inline_tensor(_np_bf16(mask_np), name="maskbd")
    e33_np = np.zeros((NL + 1, NL), np.float32)
    e33_np[:NL, :NL] = np.eye(NL)
    e33_np[NL, :] = 1.0
    e33_c = nc.inline_tensor(_np_bf16(e33_np), name="e33")
    id64_c = nc.inline_tensor(np.tile(np.eye(32, dtype=np.float32), (2, 1)), name="id64")
    id64x64_c = nc.inline_tensor(np.eye(64, dtype=np.float32), name="id64x64")
    # per-partition tanh scale for psB: rows 0-63 (g) scale 1, rows 64-127 (o) 0.5
    svec_np = np.ones((128, 1), np.float32)
    svec_np[64:128] = 0.5
    svec_c = nc.inline_tensor(svec_np, name="svec")
    z128_c = nc.inline_tensor(np.zeros((1, 128), ml_dtypes.bfloat16), name="z128")
    id128_c = nc.inline_tensor(np.eye(128, dtype=np.float32), name="id128")

    # hT column offset of h-block j under the paired [64,128] transpose layout
    def off(j):
        return 64 * (j % 4) + 32 * (j // 4)

    with PatchedTileContext(nc) as tc, ExitStack() as stack:
        persist = stack.enter_context(tc.tile_pool(name="persist", bufs=1))
        Psb = [persist.tile([128, G], dt.bfloat16, tag=f"p{m}", name=f"p{m}") for m in range(4)]
        A2bf = [persist.tile([128, NK], dt.bfloat16, tag=f"a2b{j}", name=f"a2b{j}") for j in range(8)]
        # hT split into per-half tiles so next-step matmuls can start on the
        # first transposed half while the second half's state ops still run
        hTab = [
            persist.tile([128, 128], dt.bfloat16, tag="hTa", name="hTa"),
            persist.tile([128, 128], dt.bfloat16, tag="hTb", name="hTb"),
        ]
        cfull = persist.tile([128, 512], dt.float32, tag="cfull", name="cfull")

        def hsl(j):
            """lhsT slice of h-block j: tile hx=(j%4)//2, col 64*(j%4-2*hx)+32*(j//4)."""
            hx = (j % 4) // 2
            c = 64 * (j % 4 - 2 * hx) + 32 * (j // 4)
            return hTab[hx][:, c : c + 32]

        # ============ Phase B: A2 layouts, h0T, c0, P = A2^T @ Wattn
        with (
            tc.tile_pool(name="pcb1", bufs=1) as pcb1,
            tc.tile_pool(name="pcb2", bufs=2) as pcb2,
            tc.tile_pool(name="pcbps", bufs=3, space="PSUM") as pcbps,
        ):
            A_hnk = A_d[:, :, :].rearrange("n h k -> h n k")  # [H, NL, 16]
            A2r = [pcb1.tile([128, NK], dt.float32r, tag=f"a2r{j}", name=f"a2r{j}") for j in range(8)]
            h0scr = pcb1.tile([128, 32], dt.float32, tag="h0scr", name="h0scr")
            for j in range(8):
                nc.sync.dma_start(
                    out=A2r[j].rearrange("h (n k) -> h n k", k=16),
                    in_=A_hnk[ts(j, 128), :, :].bitcast(dt.float32r),
                )
                nc.vector.tensor_copy(A2bf[j], A2r[j].bitcast(dt.float32))
                nc.vector.tensor_reduce(
                    h0scr,
                    A2r[j].bitcast(dt.float32).rearrange("h (n k) -> h n k", k=16),
                    mybir.AxisListType.X,
                    ALU.add,
                )
                nc.scalar.mul(hsl(j), h0scr, 1.0 / 16.0)
            # c0 stacked into cfull[64:128], 8 h-slices of 128
            for qq in range(8):
                a2n = pcb2.tile([NL, 128 * 16], dt.float32, tag="a2n", name="a2n")
                nc.sync.dma_start(
                    out=a2n.rearrange("n (h k) -> n h k", k=16),
                    in_=A_d[:, ts(qq, 128), :],
                )
                c0scr = pcb2.tile([NL, 128], dt.float32, tag="c0scr", name="c0scr")
                nc.vector.tensor_reduce(
                    c0scr,
                    a2n.rearrange("n (h k) -> n h k", k=16),
                    mybir.AxisListType.X,
                    ALU.add,
                )
                q, r2 = qq // 4, qq % 4
                nc.scalar.mul(
                    cfull[64 + 32 * q : 96 + 32 * q, ts(r2, 128)], c0scr, 1.0 / 8.0
                )
            # P in two Wattn halves; second half added in place (bf16)
            wat = [pcb1.tile([128, G], dt.float32r, tag=f"wat{j}", name=f"wat{j}") for j in range(4)]
            for half in range(2):
                for j in range(4):
                    nc.sync.dma_start(
                        out=wat[j],
                        in_=Wattn_d[ts(4 * half + j, 128), :].bitcast(dt.float32r),
                    )
                for m in range(4):
                    for c in range(8):
                        pps = pcbps.tile([128, 512], dt.float32, tag="pps", name="pps")
                        for j in range(4):
                            nc.tensor.matmul(
                                pps,
                                A2r[4 * half + j][:, ts(m, 128)],
                                wat[j][:, ts(c, 512)],
                                start=(j == 0),
                                stop=(j == 3),
                            )
                        if half == 0:
                            nc.scalar.copy(Psb[m][:, ts(c, 512)], pps)
                        else:
                            nc.vector.tensor_add(
                                Psb[m][:, ts(c, 512)], pps, Psb[m][:, ts(c, 512)]
                            )

        # ============ Phase A: xact = x @ Wx -> DRAM bf16 hi/lo (+ b row)
        with tc.tile_pool(name="bpool", bufs=1) as bpool:
            b_f = bpool.tile([1, G], dt.float32, tag="b_f", name="b_f")
            nc.sync.dma_start(out=b_f, in_=b_d[:, :])
            bhi = bpool.tile([1, G], dt.bfloat16, tag="bhi", name="bhi")
            nc.vector.tensor_copy(bhi, b_f)
            nc.sync.dma_start(out=xhi_d[NL * T : NL * T + 1, :], in_=bhi)

        with (
            tc.tile_pool(name="pca1", bufs=1) as pca1,
            tc.tile_pool(name="pca", bufs=2) as pca,
            tc.tile_pool(name="pcaps", bufs=3, space="PSUM") as pcaps,
        ):
            id128r = pca1.tile([128, 128], dt.float32r, tag="id128r", name="id128r")
            nc.sync.dma_start(out=id128r, in_=id128_c[:, :].bitcast(dt.float32r))
            Wxsb = [pca1.tile([128, G], dt.float32r, tag=f"wx{k}", name=f"wx{k}") for k in range(4)]
            for k in range(4):
                nc.sync.dma_start(
                    out=Wxsb[k], in_=Wx_d[ts(k, 128), :].bitcast(dt.float32r)
                )

            for i in range(NL):
                xsb = pca.tile([128, D], dt.float32r, tag="xsb", name="xsb")
                nc.sync.dma_start(out=xsb, in_=x_d[i, :, :].bitcast(dt.float32r))
                xt_ps = pcaps.tile([128, 512], dt.float32r, tag="xtps", name="xtps")
                for k in range(4):
                    nc.tensor.transpose(xt_ps[:, ts(k, 128)], xsb[:, ts(k, 128)], id128r)
                xT = pca.tile([128, 512], dt.float32r, tag="xT", name="xT")
                nc.scalar.copy(xT, xt_ps)
                for c in range(8):
                    aps = pcaps.tile([128, 512], dt.float32, tag="acps", name="acps")
                    for k in range(4):
                        nc.tensor.matmul(
                            aps,
                            xT[:, ts(k, 128)],
                            Wxsb[k][:, ts(c, 512)],
                            start=(k == 0),
                            stop=(k == 3),
                        )
                    hi = pca.tile([128, 512], dt.bfloat16, tag="hi", name="hi")
                    nc.scalar.copy(hi, aps)
                    nc.sync.dma_start(out=xhi_d[ts(i, 128), ts(c, 512)], in_=hi)

        # ============ Phase C: Wh -> bf16 resident
        whpool = stack.enter_context(tc.tile_pool(name="whpool", bufs=1))
        Whsb = [whpool.tile([128, G], dt.bfloat16, tag=f"wh{j}", name=f"wh{j}") for j in range(8)]
        with tc.tile_pool(name="pcc", bufs=2) as pcc:
            for j in range(8):
                stage = pcc.tile([128, G], dt.float32, tag="whstage", name="whstage")
                nc.sync.dma_start(out=stage, in_=Wh_d[ts(j, 128), :])
                nc.vector.tensor_copy(Whsb[j], stage)

        # ============ Recurrent loop
        loopp = stack.enter_context(tc.tile_pool(name="loopp", bufs=1))
        sbIF = loopp.tile([128, 512], dt.float32, tag="sbIF", name="sbIF")
        tB = loopp.tile([128, 512], dt.float32, tag="tB", name="tB")
        th_sb = loopp.tile([128, 512], dt.float32, tag="th_sb", name="th_sb")  # rows 64-127 used (STT base-partition rule)
        o_sb = loopp.tile([128, 512], dt.float32, tag="o_sb", name="o_sb")  # rows 64-127 used
        u_sb = loopp.tile([64, 512], dt.float32, tag="u_sb", name="u_sb")
        v_sb = loopp.tile([64, 512], dt.float32, tag="v_sb", name="v_sb")
        h_sb = loopp.tile([64, 512], dt.float32, tag="h_sb", name="h_sb")
        wsum = loopp.tile([32, 256], dt.float32, tag="wsum", name="wsum")
        svec = loopp.tile([128, 1], dt.float32, tag="svec", name="svec")
        id64x64 = loopp.tile([64, 64], dt.float32, tag="id64x64", name="id64x64")
        nc.sync.dma_start(out=svec, in_=svec_c[:, :])
        nc.sync.dma_start(out=id64x64, in_=id64x64_c[:, :])
        wexp = loopp.tile([NL, NK], dt.float32, tag="wexp", name="wexp")
        wexpn = loopp.tile([NL, NK], dt.float32, tag="wexpn", name="wexpn")
        s_sb = loopp.tile([NL, 1], dt.float32, tag="s_sb", name="s_sb")
        rs_sb = loopp.tile([NL, 1], dt.float32, tag="rs_sb", name="rs_sb")
        wBD = loopp.tile([128, 128], dt.bfloat16, tag="wBD", name="wBD")
        maskbf = loopp.tile([NL, NK], dt.bfloat16, tag="maskbf", name="maskbf")
        e33 = loopp.tile([NL + 1, NL], dt.bfloat16, tag="e33t", name="e33t")
        id64 = loopp.tile([64, 32], dt.float32, tag="id64t", name="id64t")
        z128 = loopp.tile([1, 128], dt.bfloat16, tag="z128t", name="z128t")
        nc.sync.dma_start(out=z128, in_=z128_c[:, :])
        nc.sync.dma_start(out=maskbf, in_=mask_c[:, :])
        nc.sync.dma_start(out=e33, in_=e33_c[:, :])
        nc.sync.dma_start(out=id64, in_=id64_c[:, :])

        lps = stack.enter_context(tc.tile_pool(name="lps", bufs=1, space="PSUM"))
        xpool = stack.enter_context(tc.tile_pool(name="xpool", bufs=2))
        xhi_nt = xhi_d[0 : NL * T, :].rearrange("(n t) g -> n t g", t=T)

        # chunk -> (psum tensor, block q, gate column slice)
        # psA blocks: (i,0) (i,1) (f,0) (f,1); psB blocks: (g,0) (g,1) (o,0) (o,1)
        chunk_cols = {}
        for q, (gb, half) in enumerate([(0, 0), (0, 1), (H, 0), (H, 1)]):
            chunk_cols[("A", q)] = slice(gb + 512 * half, gb + 512 * half + 512)
        for q, (gb, half) in enumerate([(3 * H, 0), (3 * H, 1), (2 * H, 0), (2 * H, 1)]):
            chunk_cols[("B", q)] = slice(gb + 512 * half, gb + 512 * half + 512)

        rep_ctx = tc.For_i(0, reps, 1) if reps > 1 else None
        if rep_ctx is not None:
            rep_ctx.__enter__()

        def make_xact(t):
            """Prefetch x-act tiles for step t and open each psum chunk with
            its one-hot x-act matmul (start=True zeroes the chunk rows; runs
            in the previous step's tail, keeping PE warm). psA/psB alternate
            between two psum banks so these opens don't WAR against the
            previous step's gate reads."""
            xhi_t = xpool.tile([NL + 1, G], dt.bfloat16, tag="xhi", name="xhi")
            nc.sync.dma_start(out=xhi_t[0:NL, :], in_=xhi_nt[:, t, :])
            nc.sync.dma_start(out=xhi_t[NL : NL + 1, :], in_=xhi_d[NL * T : NL * T + 1, :])
            pb = t % 2
            psA = lps.tile([128, 512], dt.float32, tag=f"psA{pb}", name=f"psA{pb}")
            psB = lps.tile([128, 512], dt.float32, tag=f"psB{pb}", name=f"psB{pb}")
            plan = []
            for which, ps in (("A", psA), ("B", psB)):
                for q in range(4):
                    plan.append((ps[ts(q, 32), :], (0, 32 * q), chunk_cols[(which, q)]))
            order = [0, 4, 1, 5, 2, 6, 3, 7]
            for ci in order:
                dst, tp, cs = plan[ci]
                nc.tensor.matmul(dst, e33, xhi_t[:, cs], start=True, stop=False, tile_position=tp)
            return psA, psB, plan

        nc.vector.memset(wBD, 0.0)
        cur = make_xact(0)
        # bank budget (8): psA0 psB0 psA1 psB1 psM + these three.
        # psWT shares its bank with the warm scratch (both only PE-written
        # before their single reader); psHTa/psHTb get their own banks so
        # the tail transposes/casts don't serialize on bank-overlap pairs.
        psWT_ps = lps.tile([128, 512], dt.float32, tag="psWT", name="psWT")
        warm_ps = psWT_ps[0:32, 128:256]
        psHTa = lps.tile([128, 128], dt.float32, tag="psHTa", name="psHTa")
        psHTb = lps.tile([128, 128], dt.float32, tag="psHTb", name="psHTb")

        for t in range(t_steps):
            psA, psB, plan = cur

            # ---- M-phase [32,512] on strip 0, h@Wh on strips 1-3. The mask
            # matmul runs FIRST (start=True) — it has no hT dependency, so the
            # scheduler can slot it into the previous step's gate window.
            # j-order consumes hTa then hTb, chasing the half-transposes.
            psM = lps.tile([NL, NK], dt.float32, tag="psM", name="psM")
            c123 = [1, 2, 3, 5, 6, 7]
            JORD = [0, 4, 1, 5, 2, 6, 3, 7]
            nc.tensor.matmul(psM, e33[0:NL, :], maskbf, start=True, stop=False)
            for idx, j in enumerate(JORD):
                hs = hsl(j)
                nc.tensor.matmul(psM, hs, A2bf[j], start=False, stop=(idx == 7))
                for ci in (c123[3 * (idx % 2)], c123[3 * (idx % 2) + 1], c123[3 * (idx % 2) + 2]):
                    dst, tp, cs = plan[ci]
                    nc.tensor.matmul(dst, hs, Whsb[j][:, cs], start=False, stop=False, tile_position=tp)
            # remaining h@Wh rounds (overlap the softmax chain)
            done = {(c123[3 * (idx % 2) + r], JORD[idx]) for idx in range(8) for r in range(3)}
            rest = [(ci, j) for j in range(8) for ci in [0, 4, 1, 5, 2, 6, 3, 7] if (ci, j) not in done]
            # round-robin the remainder in hTa-first order to keep streams busy
            rest.sort(key=lambda cj: (JORD.index(cj[1]), cj[0]))
            for ci, j in rest:
                dst, tp, cs = plan[ci]
                nc.tensor.matmul(dst, hsl(j), Whsb[j][:, cs], start=False, stop=False, tile_position=tp)

            # ---- softmax
            if "softmax" not in ablate:
                nc.scalar.activation(wexp, psM, AF.Exp, scale=SCALE, accum_out=s_sb)
                nc.vector.reciprocal(rs_sb, s_sb)
                nc.vector.tensor_scalar_mul(wexpn, wexp, rs_sb)
                # ---- wBD (PE transposes of normalized weights)
                psWT = psWT_ps[:, 0:128]
                for m in range(4):
                    nc.tensor.transpose(psWT[:, ts(m, 32)], wexpn[:, ts(m, 128)], id64[0:32, :])
                nc.scalar.copy(wBD, psWT)

            # ---- act matmuls part 2 (attention via P). All psA chunks run
            # first (then psA's stop), so the i/f gate tanh starts while the
            # psB half of attention still streams.
            if "attn" not in ablate:
                for m in range(4):
                    for ci in (0, 1, 2, 3):
                        dst, tp, cs = plan[ci]
                        nc.tensor.matmul(dst, wBD[:, ts(m, 32)], Psb[m][:, cs], start=False, stop=False, tile_position=tp)
                nc.tensor.matmul(psA[:, 0:1], z128, maskbf[0:1, 0:1], start=False, stop=True)
                for m in range(4):
                    for ci in (4, 5, 6, 7):
                        dst, tp, cs = plan[ci]
                        nc.tensor.matmul(dst, wBD[:, ts(m, 32)], Psb[m][:, cs], start=False, stop=False, tile_position=tp)
                nc.tensor.matmul(psB[:, 0:1], z128, maskbf[0:1, 0:1], start=False, stop=True)
            else:
                nc.tensor.matmul(psA[:, 0:1], z128, maskbf[0:1, 0:1], start=False, stop=True)
                nc.tensor.matmul(psB[:, 0:1], z128, maskbf[0:1, 0:1], start=False, stop=True)

            # ---- gates + state update, column-half pipelined.
            # sigmoid via tanh (sigma(x)=0.5+0.5*tanh(x/2)) keeps ACT on one
            # table set; the i/f affines fold into STT ops by carrying the
            # cell state doubled (cfull holds C=2c); o gets an explicit ACT
            # affine so h comes out exact (DMA'd directly).
            # Half hx covers columns 256hx:256hx+256 of every [*,512] state
            # tile, so the hx=0 transposes + next-step matmuls overlap hx=1.
            H0, H1 = slice(0, 256), slice(256, 512)
            # psA-only reads first (start during attention's psB half);
            # v = f*C also needs only psA, so it leads the DVE queue.
            nc.scalar.activation(sbIF[:, H0], psA[:, H0], AF.Tanh, scale=0.5)  # [ti; tf]
            nc.scalar.activation(sbIF[:, H1], psA[:, H1], AF.Tanh, scale=0.5)
            nc.scalar.activation(tB[:, H0], psB[:, H0], AF.Tanh, scale=svec)   # [g; to]
            nc.scalar.activation(tB[:, H1], psB[:, H1], AF.Tanh, scale=svec)
            nc.vector.scalar_tensor_tensor(v_sb[:, H0], sbIF[64:128, H0], 1.0, cfull[64:128, H0], op0=ALU.add, op1=ALU.mult)
            nc.vector.scalar_tensor_tensor(v_sb[:, H1], sbIF[64:128, H1], 1.0, cfull[64:128, H1], op0=ALU.add, op1=ALU.mult)
            # next step's x-act prefetch + psum-open runs in this tail
            if t + 1 < t_steps:
                nxt = make_xact(t + 1)
            else:
                nxt = None
            # keep-warm matmuls pinned to the DVE chain: the gate window is
            # ~4.5us of PE idle, and HAM only holds K=8/8 if the PE stays
            # busy; five N=512 warm streams (~2us busy) keep the duty cycle
            # high enough that the next step's matmul head runs at 2.4 GHz
            if "warm" not in ablate:
                nc.tensor.matmul(warm_ps, id64[0:32, :], v_sb[0:32, 0:128], start=True, stop=True)
            nc.scalar.activation(o_sb[64:128, H0], tB[64:128, H0], AF.Copy, bias=0.5, scale=0.5)
            nc.scalar.activation(o_sb[64:128, H1], tB[64:128, H1], AF.Copy, bias=0.5, scale=0.5)
            # u2 = (1+ti)*g = 2u ; C_new = u2 + 0.5*v2
            nc.vector.scalar_tensor_tensor(u_sb[:, H0], sbIF[0:64, H0], 1.0, tB[0:64, H0], op0=ALU.add, op1=ALU.mult)
            nc.vector.scalar_tensor_tensor(cfull[64:128, H0], v_sb[:, H0], 0.5, u_sb[:, H0], op0=ALU.mult, op1=ALU.add)
            nc.scalar.activation(th_sb[64:128, H0], cfull[64:128, H0], AF.Tanh, scale=0.5)
            nc.vector.scalar_tensor_tensor(u_sb[:, H1], sbIF[0:64, H1], 1.0, tB[0:64, H1], op0=ALU.add, op1=ALU.mult)
            nc.vector.scalar_tensor_tensor(cfull[64:128, H1], v_sb[:, H1], 0.5, u_sb[:, H1], op0=ALU.mult, op1=ALU.add)
            if "warm" not in ablate:
                nc.tensor.matmul(warm_ps, id64[0:32, :], u_sb[0:32, 0:128], start=True, stop=True)
            # h = o*th  [64,512]: rows 0-31 h cols 0:512, rows 32-63 h cols 512:1024
            nc.vector.tensor_mul(h_sb[:, H0], o_sb[64:128, H0], th_sb[64:128, H0])
            if t + 1 < t_steps:
                nc.tensor.transpose(psHTa[:, 0:64], h_sb[:, 0:128], id64x64)
                nc.tensor.transpose(psHTa[:, 64:128], h_sb[:, 128:256], id64x64)
            nc.scalar.activation(th_sb[64:128, H1], cfull[64:128, H1], AF.Tanh, scale=0.5)
            if t + 1 < t_steps:
                nc.vector.tensor_copy(hTab[0], psHTa)
            nc.vector.tensor_mul(h_sb[:, H1], o_sb[64:128, H1], th_sb[64:128, H1])
            if "warm" not in ablate:
                nc.tensor.matmul(warm_ps, id64[0:32, :], h_sb[0:32, 0:128], start=True, stop=True)
            if t + 1 < t_steps:
                nc.tensor.transpose(psHTb[:, 0:64], h_sb[:, 256:384], id64x64)
                nc.tensor.transpose(psHTb[:, 64:128], h_sb[:, 384:512], id64x64)
                nc.vector.tensor_copy(hTab[1], psHTb)
            nc.sync.dma_start(out=out_d[:, t, 0:512], in_=h_sb[0:32, :])
            nc.sync.dma_start(out=out_d[:, t, 512:1024], in_=h_sb[32:64, :])
            cur = nxt
        if rep_ctx is not None:
            rep_ctx.__exit__(None, None, None)
    if split:
        split_multi_waits(nc)
    return nc


_CACHE = {}


def _get_nc(t_steps):
    if t_steps not in _CACHE:
        _CACHE[t_steps] = build(t_steps)
    return _CACHE[t_steps]


def kernel(x, A, Wx, Wh, Wattn, b, t_steps=T, trace=False):
    x = np.asarray(x, np.float32)
    A = np.asarray(A, np.float32).reshape(N, H, 16)
    Wx = np.ascontiguousarray(np.asarray(Wx, np.float32))
    Wh = np.ascontiguousarray(np.asarray(Wh, np.float32))
    Wattn = np.ascontiguousarray(np.asarray(Wattn, np.float32))
    b = np.asarray(b, np.float32).reshape(1, G)

    nc = _get_nc(t_steps)
    in_maps = []
    for c in range(NCORES):
        sl = slice(NL * c, NL * (c + 1))
        in_maps.append(
            {
                "x": np.ascontiguousarray(x[sl]),
                "A": np.ascontiguousarray(A[sl]),
                "Wx": Wx,
                "Wh": Wh,
                "Wattn": Wattn,
                "b": b,
            }
        )
    res = run_bass_kernel_spmd(nc, in_maps, core_ids=list(range(NCORES)), trace=trace)
    out = np.concatenate([r["out"] for r in res.results], axis=0)
    if trace:
        kernel.last_exec_time_ns = res.exec_time_ns
    return out


kernel.last_exec_time_ns = None



# revision 52
# speedup vs baseline: 1.2519x; 1.0910x over previous
"""AttentionLSTM Trainium2 kernel (8-core SPMD, data-parallel over batch).

Problem: N=256, T=128, D=512, H=1024.
    h0 = c0 = mean(A, (2,3));  per step:
      M = einsum('nh,nhk->nk', h, A2)/sqrt(H); w = softmax(M)
      attn = einsum('nhk,nk->nh', A2, w)
      act = x_t@Wx + h@Wh + attn@Wattn + b -> i,f,o,g -> LSTM update

Per-core design (32 batch rows):
  - All recurrent matmuls in bf16, accumulated in fp32 PSUM, with PE
    column-tiling (tile_position=(0,32q)) so 4 independent M=32 matmuls
    stream concurrently.
  - attn@Wattn is algebraically folded: P[(n,k),:] = A2[n,:,k]@Wattn is
    precomputed once (f32r matmuls); per step act += wBD.T @ P where wBD is
    the block-diagonal softmax weights - attn itself never materializes.
    All psA (i/f gate) attention chunks run before psB chunks so the gate
    activations start while attention still streams.
  - M-phase uses the same diag trick: psum_M = hT.T @ A2sb (+ additive
    block-diagonal -1e30 mask via an identity matmul, issued first so it
    can slot into the previous step's gate window); one Exp activation
    with accum_out yields both exp(M/32) and its row-sum.
  - x@Wx (+b) is precomputed to DRAM in bf16; the per-step one-hot matmul
    that injects it carries start=True, doubling as the psum-open. psA/psB
    double-buffer across steps so these opens don't WAR the gate reads.
  - Gates: sigmoid(x) = 0.5+0.5*tanh(x/2) keeps the ACT engine on the
    exp/tanh table set (no ACT_TABLE_LOAD swaps); the i/f affines fold
    into fused scalar_tensor_tensor ops by carrying the cell state
    doubled (cfull = 2c); o gets an explicit affine so h is exact.
  - State update is column-half pipelined: half 0's [64,128] PE
    transposes (paired h-blocks j/j+4) and the next step's matmuls start
    while half 1's DVE/ACT chain still runs. hT lives in two per-half
    tiles (hTa/hTb).
  - Small keep-warm matmuls pinned to the DVE chain limit HAM re-throttle
    damage across the gate window.
"""
import math
from contextlib import ExitStack

import numpy as np
import ml_dtypes

import concourse.bass as bass
import concourse.mybir as mybir
import concourse.tile as tile
from concourse.bass import ts
from concourse.bass_utils import run_bass_kernel_spmd
from concourse.vector_clock import ScopedClock

dt = mybir.dt
AF = mybir.ActivationFunctionType
ALU = mybir.AluOpType

N, T, D, H = 256, 128, 512, 1024
NCORES = 8
NL = N // NCORES          # 32 batch rows per core
G = 4 * H                 # 4096 gate columns
NK = NL * 16              # 512 (n,k) pairs
SCALE = 1.0 / math.sqrt(H)


class PatchedTileContext(tile.TileContext):
    """This walrus build allows at most one sem wait per SP TPB_CTRL
    instruction; put the tail waits on single-wait NoOps before the drain."""

    def _drain_and_barrier(self, tick_clock, wait_clock):
        collector = self.nc.sync.nop(nofuse=True, hint="tail_waits")
        wait_clock.add_sem_waits(
            collector.ins, ScopedClock({None: tick_clock.global_clock})
        )
        waits = list(collector.ins.sync_info.on_wait) if collector.ins.sync_info else []
        collector.ins.sync_info = mybir.SyncInfo(on_wait=waits[:1], on_update=[])
        for w in waits[1:]:
            n = self.nc.sync.nop(nofuse=True, hint="tail_waits")
            n.ins.sync_info = mybir.SyncInfo(on_wait=[w], on_update=[])
        self.nc.sync.drain()
        self.nc.all_engine_barrier()
        assert self.sems is not None
        popped = self.nc._tile_sem_poison_stack.pop()
        assert popped is self._sem_poison
        self.nc.clear_and_free_semaphores(list(self.sems.allocated().values()))
        self.nc.all_engine_barrier()


def split_multi_waits(nc):
    """Walrus here rejects >1 sem wait per instruction: move extras onto
    same-engine NoOps inserted just before the instruction."""
    for f in nc.m.functions:
        for bb in f.blocks:
            new_insts = []
            for inst in bb.instructions:
                si = inst.sync_info
                if si is not None and len(si.on_wait) > 1:
                    waits = list(si.on_wait)
                    for w in waits[:-1]:
                        n = mybir.InstNoOp(
                            name=nc.get_next_instruction_name(),
                            engine=inst.engine,
                            ins=[],
                            outs=[],
                            sync_info=mybir.SyncInfo(on_wait=[w], on_update=[]),
                        )
                        new_insts.append(n)
                    inst.sync_info = mybir.SyncInfo(
                        on_wait=[waits[-1]], on_update=list(si.on_update)
                    )
                new_insts.append(inst)
            try:
                bb.instructions[:] = new_insts
            except TypeError:
                bb.instructions = new_insts


def _np_bf16(a):
    return a.astype(ml_dtypes.bfloat16)


def build(t_steps=T, split=True, reps=1, ablate=()):
    nc = bass.Bass("TRN2", target_bir_lowering=False, debug=False, num_devices=NCORES)

    x_d = nc.dram_tensor("x", [NL, T, D], dt.float32, kind="ExternalInput")
    A_d = nc.dram_tensor("A", [NL, H, 16], dt.float32, kind="ExternalInput")
    Wx_d = nc.dram_tensor("Wx", [D, G], dt.float32, kind="ExternalInput")
    Wh_d = nc.dram_tensor("Wh", [H, G], dt.float32, kind="ExternalInput")
    Wattn_d = nc.dram_tensor("Wattn", [H, G], dt.float32, kind="ExternalInput")
    b_d = nc.dram_tensor("b", [1, G], dt.float32, kind="ExternalInput")
    out_d = nc.dram_tensor("out", [NL, T, H], dt.float32, kind="ExternalOutput")
    # last row of each = bf16 hi/lo of the bias b
    xhi_d = nc.dram_tensor("xhi", [NL * T + 1, G], dt.bfloat16, kind="Internal")

    # ---- inline constants
    mask_np = np.full((NL, NK), -1e30, np.float32)
    for n in range(NL):
        mask_np[n, 16 * n : 16 * n + 16] = 0.0
    mask_c = nc.inline_tensor(_np_bf16(mask_np), name="maskbd")
    e33_np = np.zeros((NL + 1, NL), np.float32)
    e33_np[:NL, :NL] = np.eye(NL)
    e33_np[NL, :] = 1.0
    e33_c = nc.inline_tensor(_np_bf16(e33_np), name="e33")
    id64_c = nc.inline_tensor(np.tile(np.eye(32, dtype=np.float32), (2, 1)), name="id64")
    id64x64_c = nc.inline_tensor(np.eye(64, dtype=np.float32), name="id64x64")
    # per-partition tanh scale for psB: rows 0-63 (g) scale 1, rows 64-127 (o) 0.5
    svec_np = np.ones((128, 1), np.float32)
    svec_np[64:128] = 0.5
    svec_c = nc.inline_tensor(svec_np, name="svec")
    z128_c = nc.inline_tensor(np.zeros((1, 128), ml_dtypes.bfloat16), name="z128")
    id128_c = nc.inline_tensor(np.eye(128, dtype=np.float32), name="id128")

    # hT column offset of h-block j under the paired [64,128] transpose layout
    def off(j):
        return 64 * (j % 4) + 32 * (j // 4)

    with PatchedTileContext(nc) as tc, ExitStack() as stack:
        persist = stack.enter_context(tc.tile_pool(name="persist", bufs=1))
        Psb = [persist.tile([128, G], dt.bfloat16, tag=f"p{m}", name=f"p{m}") for m in range(4)]
        A2bf = [persist.tile([128, NK], dt.bfloat16, tag=f"a2b{j}", name=f"a2b{j}") for j in range(8)]
        # hT split into per-half tiles so next-step matmuls can start on the
        # first transposed half while the second half's state ops still run
        hTab = [
            persist.tile([128, 128], dt.bfloat16, tag="hTa", name="hTa"),
            persist.tile([128, 128], dt.bfloat16, tag="hTb", name="hTb"),
        ]
        cfull = persist.tile([128, 512], dt.float32, tag="cfull", name="cfull")

        def hsl(j):
            """lhsT slice of h-block j: tile hx=(j%4)//2, col 64*(j%4-2*hx)+32*(j//4)."""
            hx = (j % 4) // 2
            c = 64 * (j % 4 - 2 * hx) + 32 * (j // 4)
            return hTab[hx][:, c : c + 32]

        # ============ Phase B: A2 layouts, h0T, c0, P = A2^T @ Wattn
        with (
            tc.tile_pool(name="pcb1", bufs=1) as pcb1,
            tc.tile_pool(name="pcb2", bufs=2) as pcb2,
            tc.tile_pool(name="pcbps", bufs=3, space="PSUM") as pcbps,
        ):
            A_hnk = A_d[:, :, :].rearrange("n h k -> h n k")  # [H, NL, 16]
            A2r = [pcb1.tile([128, NK], dt.float32r, tag=f"a2r{j}", name=f"a2r{j}") for j in range(8)]
            h0scr = pcb1.tile([128, 32], dt.float32, tag="h0scr", name="h0scr")
            for j in range(8):
                nc.sync.dma_start(
                    out=A2r[j].rearrange("h (n k) -> h n k", k=16),
                    in_=A_hnk[ts(j, 128), :, :].bitcast(dt.float32r),
                )
                nc.vector.tensor_copy(A2bf[j], A2r[j].bitcast(dt.float32))
                nc.vector.tensor_reduce(
                    h0scr,
                    A2r[j].bitcast(dt.float32).rearrange("h (n k) -> h n k", k=16),
                    mybir.AxisListType.X,
                    ALU.add,
                )
                nc.scalar.mul(hsl(j), h0scr, 1.0 / 16.0)
            # c0 stacked into cfull[64:128], 8 h-slices of 128
            for qq in range(8):
                a2n = pcb2.tile([NL, 128 * 16], dt.float32, tag="a2n", name="a2n")
                nc.sync.dma_start(
                    out=a2n.rearrange("n (h k) -> n h k", k=16),
                    in_=A_d[:, ts(qq, 128), :],
                )
                c0scr = pcb2.tile([NL, 128], dt.float32, tag="c0scr", name="c0scr")
                nc.vector.tensor_reduce(
                    c0scr,
                    a2n.rearrange("n (h k) -> n h k", k=16),
                    mybir.AxisListType.X,
                    ALU.add,
                )
                q, r2 = qq // 4, qq % 4
                nc.scalar.mul(
                    cfull[64 + 32 * q : 96 + 32 * q, ts(r2, 128)], c0scr, 1.0 / 8.0
                )
            # P in two Wattn halves; second half added in place (bf16)
            wat = [pcb1.tile([128, G], dt.float32r, tag=f"wat{j}", name=f"wat{j}") for j in range(4)]
            for half in range(2):
                for j in range(4):
                    nc.sync.dma_start(
                        out=wat[j],
                        in_=Wattn_d[ts(4 * half + j, 128), :].bitcast(dt.float32r),
                    )
                for m in range(4):
                    for c in range(8):
                        pps = pcbps.tile([128, 512], dt.float32, tag="pps", name="pps")
                        for j in range(4):
                            nc.tensor.matmul(
                                pps,
                                A2r[4 * half + j][:, ts(m, 128)],
                                wat[j][:, ts(c, 512)],
                                start=(j == 0),
                                stop=(j == 3),
                            )
                        if half == 0:
                            nc.scalar.copy(Psb[m][:, ts(c, 512)], pps)
                        else:
                            nc.vector.tensor_add(
                                Psb[m][:, ts(c, 512)], pps, Psb[m][:, ts(c, 512)]
                            )

        # ============ Phase A: xact = x @ Wx -> DRAM bf16 hi/lo (+ b row)
        with tc.tile_pool(name="bpool", bufs=1) as bpool:
            b_f = bpool.tile([1, G], dt.float32, tag="b_f", name="b_f")
            nc.sync.dma_start(out=b_f, in_=b_d[:, :])
            bhi = bpool.tile([1, G], dt.bfloat16, tag="bhi", name="bhi")
            nc.vector.tensor_copy(bhi, b_f)
            nc.sync.dma_start(out=xhi_d[NL * T : NL * T + 1, :], in_=bhi)

        with (
            tc.tile_pool(name="pca1", bufs=1) as pca1,
            tc.tile_pool(name="pca", bufs=4) as pca,
            tc.tile_pool(name="pcaps", bufs=4, space="PSUM") as pcaps,
        ):
            id128r = pca1.tile([128, 128], dt.float32r, tag="id128r", name="id128r")
            nc.sync.dma_start(out=id128r, in_=id128_c[:, :].bitcast(dt.float32r))
            Wxsb = [pca1.tile([128, G], dt.float32r, tag=f"wx{k}", name=f"wx{k}") for k in range(4)]
            for k in range(4):
                nc.sync.dma_start(
                    out=Wxsb[k], in_=Wx_d[ts(k, 128), :].bitcast(dt.float32r)
                )

            for i in range(NL):
                xsb = pca.tile([128, D], dt.float32r, tag="xsb", name="xsb")
                nc.sync.dma_start(out=xsb, in_=x_d[i, :, :].bitcast(dt.float32r))
                xt_ps = pcaps.tile([128, 512], dt.float32r, tag="xtps", name="xtps")
                for k in range(4):
                    nc.tensor.transpose(xt_ps[:, ts(k, 128)], xsb[:, ts(k, 128)], id128r)
                xT = pca.tile([128, 512], dt.float32r, tag="xT", name="xT")
                nc.scalar.copy(xT, xt_ps)
                for c in range(8):
                    aps = pcaps.tile([128, 512], dt.float32, tag="acps", name="acps")
                    for k in range(4):
                        nc.tensor.matmul(
                            aps,
                            xT[:, ts(k, 128)],
                            Wxsb[k][:, ts(c, 512)],
                            start=(k == 0),
                            stop=(k == 3),
                        )
                    hi = pca.tile([128, 512], dt.bfloat16, tag="hi", name="hi")
                    nc.scalar.copy(hi, aps)
                    nc.sync.dma_start(out=xhi_d[ts(i, 128), ts(c, 512)], in_=hi)

        # ============ Phase C: Wh -> bf16 resident
        whpool = stack.enter_context(tc.tile_pool(name="whpool", bufs=1))
        Whsb = [whpool.tile([128, G], dt.bfloat16, tag=f"wh{j}", name=f"wh{j}") for j in range(8)]
        with tc.tile_pool(name="pcc", bufs=2) as pcc:
            for j in range(8):
                stage = pcc.tile([128, G], dt.float32, tag="whstage", name="whstage")
                nc.sync.dma_start(out=stage, in_=Wh_d[ts(j, 128), :])
                nc.vector.tensor_copy(Whsb[j], stage)

        # ============ Recurrent loop
        loopp = stack.enter_context(tc.tile_pool(name="loopp", bufs=1))
        sbIF = loopp.tile([128, 512], dt.float32, tag="sbIF", name="sbIF")
        tB = loopp.tile([128, 512], dt.float32, tag="tB", name="tB")
        th_sb = loopp.tile([128, 512], dt.float32, tag="th_sb", name="th_sb")  # rows 64-127 used (STT base-partition rule)
        o_sb = loopp.tile([128, 512], dt.float32, tag="o_sb", name="o_sb")  # rows 64-127 used
        u_sb = loopp.tile([64, 512], dt.float32, tag="u_sb", name="u_sb")
        v_sb = loopp.tile([64, 512], dt.float32, tag="v_sb", name="v_sb")
        h_sb = loopp.tile([64, 512], dt.float32, tag="h_sb", name="h_sb")
        wsum = loopp.tile([32, 256], dt.float32, tag="wsum", name="wsum")
        svec = loopp.tile([128, 1], dt.float32, tag="svec", name="svec")
        id64x64 = loopp.tile([64, 64], dt.float32, tag="id64x64", name="id64x64")
        nc.sync.dma_start(out=svec, in_=svec_c[:, :])
        nc.sync.dma_start(out=id64x64, in_=id64x64_c[:, :])
        wexp = loopp.tile([NL, NK], dt.float32, tag="wexp", name="wexp")
        wexpn = loopp.tile([NL, NK], dt.float32, tag="wexpn", name="wexpn")
        s_sb = loopp.tile([NL, 1], dt.float32, tag="s_sb", name="s_sb")
        rs_sb = loopp.tile([NL, 1], dt.float32, tag="rs_sb", name="rs_sb")
        wBD = loopp.tile([128, 128], dt.bfloat16, tag="wBD", name="wBD")
        maskbf = loopp.tile([NL, NK], dt.bfloat16, tag="maskbf", name="maskbf")
        e33 = loopp.tile([NL + 1, NL], dt.bfloat16, tag="e33t", name="e33t")
        id64 = loopp.tile([64, 32], dt.float32, tag="id64t", name="id64t")
        z128 = loopp.tile([1, 128], dt.bfloat16, tag="z128t", name="z128t")
        nc.sync.dma_start(out=z128, in_=z128_c[:, :])
        nc.sync.dma_start(out=maskbf, in_=mask_c[:, :])
        nc.sync.dma_start(out=e33, in_=e33_c[:, :])
        nc.sync.dma_start(out=id64, in_=id64_c[:, :])

        lps = stack.enter_context(tc.tile_pool(name="lps", bufs=1, space="PSUM"))
        xpool = stack.enter_context(tc.tile_pool(name="xpool", bufs=2))
        xhi_nt = xhi_d[0 : NL * T, :].rearrange("(n t) g -> n t g", t=T)

        # chunk -> (psum tensor, block q, gate column slice)
        # psA blocks: (i,0) (i,1) (f,0) (f,1); psB blocks: (g,0) (g,1) (o,0) (o,1)
        chunk_cols = {}
        for q, (gb, half) in enumerate([(0, 0), (0, 1), (H, 0), (H, 1)]):
            chunk_cols[("A", q)] = slice(gb + 512 * half, gb + 512 * half + 512)
        for q, (gb, half) in enumerate([(3 * H, 0), (3 * H, 1), (2 * H, 0), (2 * H, 1)]):
            chunk_cols[("B", q)] = slice(gb + 512 * half, gb + 512 * half + 512)

        rep_ctx = tc.For_i(0, reps, 1) if reps > 1 else None
        if rep_ctx is not None:
            rep_ctx.__enter__()

        def make_xact(t):
            """Prefetch x-act tiles for step t and open each psum chunk with
            its one-hot x-act matmul (start=True zeroes the chunk rows; runs
            in the previous step's tail, keeping PE warm). psA/psB alternate
            between two psum banks so these opens don't WAR against the
            previous step's gate reads."""
            xhi_t = xpool.tile([NL + 1, G], dt.bfloat16, tag="xhi", name="xhi")
            nc.sync.dma_start(out=xhi_t[0:NL, :], in_=xhi_nt[:, t, :])
            nc.sync.dma_start(out=xhi_t[NL : NL + 1, :], in_=xhi_d[NL * T : NL * T + 1, :])
            pb = t % 2
            psA = lps.tile([128, 512], dt.float32, tag=f"psA{pb}", name=f"psA{pb}")
            psB = lps.tile([128, 512], dt.float32, tag=f"psB{pb}", name=f"psB{pb}")
            plan = []
            for which, ps in (("A", psA), ("B", psB)):
                for q in range(4):
                    plan.append((ps[ts(q, 32), :], (0, 32 * q), chunk_cols[(which, q)]))
            order = [0, 4, 1, 5, 2, 6, 3, 7]
            for ci in order:
                dst, tp, cs = plan[ci]
                nc.tensor.matmul(dst, e33, xhi_t[:, cs], start=True, stop=False, tile_position=tp)
            return psA, psB, plan

        nc.vector.memset(wBD, 0.0)
        cur = make_xact(0)
        # bank budget (8): psA0 psB0 psA1 psB1 psM + these three.
        # psWT shares its bank with the warm scratch (both only PE-written
        # before their single reader); psHTa/psHTb get their own banks so
        # the tail transposes/casts don't serialize on bank-overlap pairs.
        psWT_ps = lps.tile([128, 512], dt.float32, tag="psWT", name="psWT")
        warm_ps = psWT_ps[0:32, 128:256]
        psHTa = lps.tile([128, 128], dt.float32, tag="psHTa", name="psHTa")
        psHTb = lps.tile([128, 128], dt.float32, tag="psHTb", name="psHTb")

        for t in range(t_steps):
            psA, psB, plan = cur

            # ---- M-phase [32,512] on strip 0, h@Wh on strips 1-3. The mask
            # matmul runs FIRST (start=True) — it has no hT dependency, so the
            # scheduler can slot it into the previous step's gate window.
            # j-order consumes hTa then hTb, chasing the half-transposes.
            psM = lps.tile([NL, NK], dt.float32, tag="psM", name="psM")
            c123 = [1, 2, 3, 5, 6, 7]
            JORD = [0, 4, 1, 5, 2, 6, 3, 7]
            nc.tensor.matmul(psM, e33[0:NL, :], maskbf, start=True, stop=False)
            for idx, j in enumerate(JORD):
                hs = hsl(j)
                nc.tensor.matmul(psM, hs, A2bf[j], start=False, stop=(idx == 7))
                for ci in (c123[3 * (idx % 2)], c123[3 * (idx % 2) + 1], c123[3 * (idx % 2) + 2]):
                    dst, tp, cs = plan[ci]
                    nc.tensor.matmul(dst, hs, Whsb[j][:, cs], start=False, stop=False, tile_position=tp)
            # remaining h@Wh rounds (overlap the softmax chain)
            done = {(c123[3 * (idx % 2) + r], JORD[idx]) for idx in range(8) for r in range(3)}
            rest = [(ci, j) for j in range(8) for ci in [0, 4, 1, 5, 2, 6, 3, 7] if (ci, j) not in done]
            # round-robin the remainder in hTa-first order to keep streams busy
            rest.sort(key=lambda cj: (JORD.index(cj[1]), cj[0]))
            for ci, j in rest:
                dst, tp, cs = plan[ci]
                nc.tensor.matmul(dst, hsl(j), Whsb[j][:, cs], start=False, stop=False, tile_position=tp)

            # ---- softmax
            if "softmax" not in ablate:
                nc.scalar.activation(wexp, psM, AF.Exp, scale=SCALE, accum_out=s_sb)
                nc.vector.reciprocal(rs_sb, s_sb)
                nc.vector.tensor_scalar_mul(wexpn, wexp, rs_sb)
                # ---- wBD (PE transposes of normalized weights)
                psWT = psWT_ps[:, 0:128]
                for m in range(4):
                    nc.tensor.transpose(psWT[:, ts(m, 32)], wexpn[:, ts(m, 128)], id64[0:32, :])
                nc.scalar.copy(wBD, psWT)

            # ---- act matmuls part 2 (attention via P). All psA chunks run
            # first (then psA's stop), so the i/f gate tanh starts while the
            # psB half of attention still streams.
            if "attn" not in ablate:
                for m in range(4):
                    for ci in (0, 1, 2, 3):
                        dst, tp, cs = plan[ci]
                        nc.tensor.matmul(dst, wBD[:, ts(m, 32)], Psb[m][:, cs], start=False, stop=False, tile_position=tp)
                nc.tensor.matmul(psA[:, 0:1], z128, maskbf[0:1, 0:1], start=False, stop=True)
                for m in range(4):
                    for ci in (4, 5, 6, 7):
                        dst, tp, cs = plan[ci]
                        nc.tensor.matmul(dst, wBD[:, ts(m, 32)], Psb[m][:, cs], start=False, stop=False, tile_position=tp)
                nc.tensor.matmul(psB[:, 0:1], z128, maskbf[0:1, 0:1], start=False, stop=True)
            else:
                nc.tensor.matmul(psA[:, 0:1], z128, maskbf[0:1, 0:1], start=False, stop=True)
                nc.tensor.matmul(psB[:, 0:1], z128, maskbf[0:1, 0:1], start=False, stop=True)

            # ---- gates + state update, column-half pipelined.
            # sigmoid via tanh (sigma(x)=0.5+0.5*tanh(x/2)) keeps ACT on one
            # table set; the i/f affines fold into STT ops by carrying the
            # cell state doubled (cfull holds C=2c); o gets an explicit ACT
            # affine so h comes out exact (DMA'd directly).
            # Half hx covers columns 256hx:256hx+256 of every [*,512] state
            # tile, so the hx=0 transposes + next-step matmuls overlap hx=1.
            H0, H1 = slice(0, 256), slice(256, 512)
            # psA-only reads first (start during attention's psB half);
            # v = f*C also needs only psA, so it leads the DVE queue.
            nc.scalar.activation(sbIF[:, H0], psA[:, H0], AF.Tanh, scale=0.5)  # [ti; tf]
            nc.scalar.activation(sbIF[:, H1], psA[:, H1], AF.Tanh, scale=0.5)
            nc.scalar.activation(tB[:, H0], psB[:, H0], AF.Tanh, scale=svec)   # [g; to]
            nc.scalar.activation(tB[:, H1], psB[:, H1], AF.Tanh, scale=svec)
            nc.vector.scalar_tensor_tensor(v_sb[:, H0], sbIF[64:128, H0], 1.0, cfull[64:128, H0], op0=ALU.add, op1=ALU.mult)
            nc.vector.scalar_tensor_tensor(v_sb[:, H1], sbIF[64:128, H1], 1.0, cfull[64:128, H1], op0=ALU.add, op1=ALU.mult)
            # next step's x-act prefetch + psum-open runs in this tail
            if t + 1 < t_steps:
                nxt = make_xact(t + 1)
            else:
                nxt = None
            # keep-warm matmuls pinned to the DVE chain: the gate window is
            # ~4.5us of PE idle, and HAM only holds K=8/8 if the PE stays
            # busy; five N=512 warm streams (~2us busy) keep the duty cycle
            # high enough that the next step's matmul head runs at 2.4 GHz
            if "warm" not in ablate:
                nc.tensor.matmul(warm_ps, id64[0:32, :], v_sb[0:32, 0:128], start=True, stop=True)
            nc.scalar.activation(o_sb[64:128, H0], tB[64:128, H0], AF.Copy, bias=0.5, scale=0.5)
            nc.scalar.activation(o_sb[64:128, H1], tB[64:128, H1], AF.Copy, bias=0.5, scale=0.5)
            # u2 = (1+ti)*g = 2u ; C_new = u2 + 0.5*v2
            nc.vector.scalar_tensor_tensor(u_sb[:, H0], sbIF[0:64, H0], 1.0, tB[0:64, H0], op0=ALU.add, op1=ALU.mult)
            nc.vector.scalar_tensor_tensor(cfull[64:128, H0], v_sb[:, H0], 0.5, u_sb[:, H0], op0=ALU.mult, op1=ALU.add)
            nc.scalar.activation(th_sb[64:128, H0], cfull[64:128, H0], AF.Tanh, scale=0.5)
            nc.vector.scalar_tensor_tensor(u_sb[:, H1], sbIF[0:64, H1], 1.0, tB[0:64, H1], op0=ALU.add, op1=ALU.mult)
            nc.vector.scalar_tensor_tensor(cfull[64:128, H1], v_sb[:, H1], 0.5, u_sb[:, H1], op0=ALU.mult, op1=ALU.add)
            if "warm" not in ablate:
                nc.tensor.matmul(warm_ps, id64[0:32, :], u_sb[0:32, 0:128], start=True, stop=True)
            # h = o*th  [64,512]: rows 0-31 h cols 0:512, rows 32-63 h cols 512:1024
            nc.vector.tensor_mul(h_sb[:, H0], o_sb[64:128, H0], th_sb[64:128, H0])
            if t + 1 < t_steps:
                nc.tensor.transpose(psHTa[:, 0:64], h_sb[:, 0:128], id64x64)
                nc.tensor.transpose(psHTa[:, 64:128], h_sb[:, 128:256], id64x64)
            nc.scalar.activation(th_sb[64:128, H1], cfull[64:128, H1], AF.Tanh, scale=0.5)
            if t + 1 < t_steps:
                nc.vector.tensor_copy(hTab[0], psHTa)
            nc.vector.tensor_mul(h_sb[:, H1], o_sb[64:128, H1], th_sb[64:128, H1])
            if "warm" not in ablate:
                nc.tensor.matmul(warm_ps, id64[0:32, :], h_sb[0:32, 0:128], start=True, stop=True)
            if t + 1 < t_steps:
                nc.tensor.transpose(psHTb[:, 0:64], h_sb[:, 256:384], id64x64)
                nc.tensor.transpose(psHTb[:, 64:128], h_sb[:, 384:512], id64x64)
                nc.vector.tensor_copy(hTab[1], psHTb)
            nc.sync.dma_start(out=out_d[:, t, 0:512], in_=h_sb[0:32, :])
            nc.sync.dma_start(out=out_d[:, t, 512:1024], in_=h_sb[32:64, :])
            cur = nxt
        if rep_ctx is not None:
            rep_ctx.__exit__(None, None, None)
    if split:
        split_multi_waits(nc)
    return nc


_CACHE = {}


def _get_nc(t_steps):
    if t_steps not in _CACHE:
        _CACHE[t_steps] = build(t_steps)
    return _CACHE[t_steps]


def kernel(x, A, Wx, Wh, Wattn, b, t_steps=T, trace=False):
    x = np.asarray(x, np.float32)
    A = np.asarray(A, np.float32).reshape(N, H, 16)
    Wx = np.ascontiguousarray(np.asarray(Wx, np.float32))
    Wh = np.ascontiguousarray(np.asarray(Wh, np.float32))
    Wattn = np.ascontiguousarray(np.asarray(Wattn, np.float32))
    b = np.asarray(b, np.float32).reshape(1, G)

    nc = _get_nc(t_steps)
    in_maps = []
    for c in range(NCORES):
        sl = slice(NL * c, NL * (c + 1))
        in_maps.append(
            {
                "x": np.ascontiguousarray(x[sl]),
                "A": np.ascontiguousarray(A[sl]),
                "Wx": Wx,
                "Wh": Wh,
                "Wattn": Wattn,
                "b": b,
            }
        )
    res = run_bass_kernel_spmd(nc, in_maps, core_ids=list(range(NCORES)), trace=trace)
    out = np.concatenate([r["out"] for r in res.results], axis=0)
    if trace:
        kernel.last_exec_time_ns = res.exec_time_ns
    return out


kernel.last_exec_time_ns = None

